# revision 20
# baseline (speedup 1.0000x reference)
"""LIF spike-train scan (nn_LIFSpike) on 8 TRN2 NeuronCores.

Reference semantics (fp32, bit-exact):
    u_t = TAU * u_{t-1} * (1 - o_{t-1}) + x_t ;  o_t = (u_t > VTH)
with u_{-1} = o_{-1} = 0, scanned over the trailing time dim (T=50).

Sharding: pure data parallel - the 16*64*32*32 = 1,048,576 spatial elements
split evenly across 8 cores (131,072 = 128 partitions x 1024 each).

On-chip layout per core: the time axis is chunked (TC-step chunks, ending
with TAIL1 single-step chunks that shorten the serial end-of-kernel tail);
each chunk tile is [128 partitions, tc, 1024] so every compute instruction
covers the full 1024-element free dim (amortizes the cayman per-instruction
read-write bubble).  The membrane history for a chunk lives in SBUF, so the
spike threshold runs as ONE is_gt instruction per chunk over [128, tc*1024].
Spikes are written as uint8 {0,1} (exact) to quarter the output HBM traffic;
the host converts back to f32.  x-in DMAs issue on the SP HW-DGE ring and
o-out DMAs on the ACT ring (first chunk's two x slices go to both rings so
the fill isn't serialized), with 4/3/4-deep tile pools for full overlap.

Per step the membrane update is one fused DVE op:
    u_t = select(VTH >= u_{t-1}, u_{t-1}, 0) * TAU + x_t
which reproduces the reference rounding exactly: round(TAU*u) then *{0,1}
then round(+x) == round(TAU*(u*{0,1})) + x for each branch.  The spike
compare is a strict is_gt (no activation-table approximations anywhere).

All compute is on the Vector (DVE) engine; nothing runs on gpsimd (Q7
software loops are ~15ns/element - two orders of magnitude off DVE).
"""

import os
import numpy as np

import concourse.bass as bass
import concourse.bacc as bacc
import concourse.tile as tile
from concourse import mybir
from concourse.bass_utils import run_bass_kernel_spmd

TAU = 0.3
VTH = 0.3

T = 50
S_FULL = 16 * 64 * 32 * 32          # 1,048,576 spatial elements
N_CORES = 8
S_CORE = S_FULL // N_CORES          # 131,072
P = 128                             # SBUF partitions
F = S_CORE // P                     # 1024 spatial elements per partition

TC = int(os.environ.get("LIF_TC", "2"))             # time-steps per chunk
NC = T // TC                                        # chunks (must divide T)
SPIKE_CHUNK = os.environ.get("LIF_SPIKE_CHUNK", "1") == "1"
O_DT = os.environ.get("LIF_O_DT", "u8")             # u8 | bf16 | f32
X_BUFS = int(os.environ.get("LIF_X_BUFS", "4"))
U_BUFS = int(os.environ.get("LIF_U_BUFS", "3"))
O_BUFS = int(os.environ.get("LIF_O_BUFS", "4"))
# DMA issue queues: sync | scalar | alt (alternate per chunk across both
# HW-DGE rings so neither sequencer saturates)
DMA_Q = os.environ.get("LIF_DMA_Q", "dir")
X_SPLIT = int(os.environ.get("LIF_X_SPLIT", "2"))   # x-DMA slices per chunk
O_SPLIT = int(os.environ.get("LIF_O_SPLIT", "1"))   # o-DMA slices per chunk
# Shorten the serial end-of-kernel tail (last x-DMA -> fused -> spike -> o-DMA)
# by finishing with single-step chunks.
TAIL1 = int(os.environ.get("LIF_TAIL1", "2"))       # trailing TC=1 chunks
assert T % TC == 0 and TC % X_SPLIT == 0 and TC % O_SPLIT == 0
assert TAIL1 % TC == 0 and TAIL1 < T
# chunk schedule: uniform TC chunks, then TAIL1 single-step chunks
CHUNKS = [TC] * ((T - TAIL1) // TC) + [1] * TAIL1
N2 = (T - TAIL1) // TC                              # count of TC-sized chunks

# results of the last run (for test.py to inspect trace/exec time)
LAST_RESULTS = None

_FUSED_OP = None


def _get_fused_op():
    """Register the fused gated-leak op: out = select(VTH >= u, u, 0)*TAU + x.

    One DVE instruction per scan step instead of two scalar_tensor_tensor
    passes.  Registered at runtime into concourse.dve_ops' module-level
    registry (OPS / CUSTOM_DVE_SPECS / opcode map), which is all the
    table-gen path reads."""
    global _FUSED_OP
    if _FUSED_OP is not None:
        return _FUSED_OP
    import concourse.dve_ops as dve_ops
    from concourse.dve_spec import Spec, Src0, Src1, C0, C1, Zero, select, lower
    from concourse.dve_uop import DveOpSpec

    name = "LIF_GATED_LEAK_ANT"
    spec = Spec(
        body=select(C0 >= Src0, Src0, Zero) * C1 + Src1,
        reference=lambda in0, in1, s0, s1, imm2: (
            np.where(s0 >= in0, in0, np.float32(0.0)).astype(np.float32) * np.float32(s1)
        ).astype(np.float32)
        + in1,
    )
    existing = {op.name for op in dve_ops.OPS}
    if name not in existing:
        row = dve_ops._CUSTOM_DVE_ROW_BASE + len(dve_ops.OPS)
        assert row < 0x20, "custom-DVE opcode row overflow"
        # pin the sha to what lower() actually produces (self-consistent)
        shas = {}
        for ver in ("v3", "v4"):
            uops = lower(spec, ver=ver)
            shas[ver] = DveOpSpec(name=name, opcode=row, uops=uops, rd1_en=True).sha(ver)
        op = dve_ops.DveOp(name, spec, subdim=False, uops_sha=shas)
        dve_ops.OPS.append(op)
        dve_ops.CUSTOM_DVE_SPECS[name] = spec
        dve_ops._SUB_OPCODE_FOR_NAME[name] = row
        _FUSED_OP = op
    else:
        _FUSED_OP = next(op for op in dve_ops.OPS if op.name == name)
    return _FUSED_OP


def _o_mybir_dt():
    return {
        "u8": mybir.dt.uint8,
        "bf16": mybir.dt.bfloat16,
        "f32": mybir.dt.float32,
    }[O_DT]


def _build_program():
    f32 = mybir.dt.float32
    odt = _o_mybir_dt()
    nc = bacc.Bacc("TRN2", target_bir_lowering=False, debug=False)

    x_d2 = nc.dram_tensor("x2", [N2, P, TC, F], f32, kind="ExternalInput").ap()
    o_d2 = nc.dram_tensor("o2", [N2, P, TC, F], odt, kind="ExternalOutput").ap()
    if TAIL1:
        x_d1 = nc.dram_tensor("x1", [TAIL1, P, 1, F], f32, kind="ExternalInput").ap()
        o_d1 = nc.dram_tensor("o1", [TAIL1, P, 1, F], odt, kind="ExternalOutput").ap()

    fused = _get_fused_op()

    with tile.TileContext(nc) as tc:
        with (
            tc.tile_pool(name="xp", bufs=X_BUFS) as xp,
            tc.tile_pool(name="up", bufs=U_BUFS) as up,
            tc.tile_pool(name="op", bufs=O_BUFS) as op_,
        ):
            def dma_eng(idx, out=False):
                if DMA_Q == "sync":
                    return nc.sync
                if DMA_Q == "scalar":
                    return nc.scalar
                if DMA_Q == "dir":  # x-in on SP ring, o-out on ACT ring
                    return nc.scalar if out else nc.sync
                return nc.sync if idx % 2 == 0 else nc.scalar

            u_prev = None  # [P, F] slice of the previous chunk's history
            t0 = 0
            for c, tcn in enumerate(CHUNKS):
                xin = x_d2[c] if c < N2 else x_d1[c - N2]
                oout = o_d2[c] if c < N2 else o_d1[c - N2]
                xt = xp.tile([P, tcn, F], f32)
                nspl = X_SPLIT if tcn % X_SPLIT == 0 else 1
                xs = tcn // nspl
                for s in range(nspl):
                    # first chunk: land the two slices via both HW-DGE rings
                    # concurrently so fill isn't serialized on one ring
                    eng = (nc.sync if s == 0 else nc.scalar) if c == 0 \
                        else dma_eng(c)
                    eng.dma_start(
                        out=xt[:, s * xs:(s + 1) * xs, :],
                        in_=xin[:, s * xs:(s + 1) * xs, :],
                    )
                uh = up.tile([P, tcn, F], f32)  # membrane history for chunk
                ot = op_.tile([P, tcn, F], odt)

                for tl in range(tcn):
                    u_new = uh[:, tl, :]
                    if c == 0 and tl == 0:
                        # u_0 = x_0 (zero carry)
                        nc.vector.tensor_copy(u_new, xt[:, 0, :])
                    else:
                        nc.vector._custom_dve(
                            fused,
                            out=u_new,
                            in0=u_prev,
                            in1=xt[:, tl, :],
                            s0=VTH,
                            s1=TAU,
                        )
                    u_prev = u_new
                    if not SPIKE_CHUNK:
                        nc.vector.tensor_scalar(
                            ot[:, tl, :], u_new, VTH, None, mybir.AluOpType.is_gt
                        )
                if SPIKE_CHUNK:
                    # one strict-compare over the whole chunk history
                    nc.vector.tensor_scalar(
                        ot[:], uh[:], VTH, None, mybir.AluOpType.is_gt
                    )
                nspl = O_SPLIT if tcn % O_SPLIT == 0 else 1
                os_ = tcn // nspl
                for s in range(nspl):
                    dma_eng(c + 1, out=True).dma_start(
                        out=oout[:, s * os_:(s + 1) * os_, :],
                        in_=ot[:, s * os_:(s + 1) * os_, :],
                    )
                t0 += tcn
    nc.compile()
    return nc


def kernel(x, ksi=None, trace=False):
    """Full-input entry: x [16,64,32,32,50] f32 -> spikes, same shape.
    (ksi is unused by the reference computation.)"""
    global LAST_RESULTS
    x = np.ascontiguousarray(np.asarray(x, dtype=np.float32))
    orig_shape = x.shape
    xf = x.reshape(S_FULL, T)

    nc = _build_program()

    # device layout per core: [chunk, partition, t-in-chunk, free-spatial]
    T2 = T - TAIL1
    in_maps = []
    for i in range(N_CORES):
        xc = xf[i * S_CORE:(i + 1) * S_CORE]            # [S_CORE, T]
        x2 = xc[:, :T2].reshape(P, F, N2, TC).transpose(2, 0, 3, 1)
        m = {"x2": np.ascontiguousarray(x2)}            # [N2, P, TC, F]
        if TAIL1:
            x1 = xc[:, T2:].reshape(P, F, TAIL1, 1).transpose(2, 0, 3, 1)
            m["x1"] = np.ascontiguousarray(x1)          # [TAIL1, P, 1, F]
        in_maps.append(m)

    res = run_bass_kernel_spmd(nc, in_maps, list(range(N_CORES)), trace=trace)
    LAST_RESULTS = res

    def decode(r, nch, tcn, tspan):
        oc = np.asarray(r).transpose(1, 3, 0, 2).reshape(S_CORE, tspan)
        if oc.dtype != np.float32:
            oc = (oc != 0).astype(np.float32) if O_DT == "u8" else oc.astype(np.float32)
        return oc

    out = np.empty((S_FULL, T), dtype=np.float32)
    for i in range(N_CORES):
        sl = out[i * S_CORE:(i + 1) * S_CORE]
        sl[:, :T2] = decode(res.results[i]["o2"], N2, TC, T2)
        if TAIL1:
            sl[:, T2:] = decode(res.results[i]["o1"], TAIL1, 1, TAIL1)
    return out.reshape(orig_shape)


# revision 23
# speedup vs baseline: 1.0020x; 1.0020x over previous
"""LIF spike-train scan (nn_LIFSpike) on 8 TRN2 NeuronCores.

Reference semantics (fp32, bit-exact):
    u_t = TAU * u_{t-1} * (1 - o_{t-1}) + x_t ;  o_t = (u_t > VTH)
with u_{-1} = o_{-1} = 0, scanned over the trailing time dim (T=50).

Sharding: pure data parallel - the 16*64*32*32 = 1,048,576 spatial elements
split evenly across 8 cores (131,072 = 128 partitions x 1024 each).

On-chip layout per core: the time axis is chunked (TC-step chunks, ending
with TAIL1 single-step chunks that shorten the serial end-of-kernel tail);
each chunk tile is [128 partitions, tc, 1024] so every compute instruction
covers the full 1024-element free dim (amortizes the cayman per-instruction
read-write bubble).  The membrane history for a chunk lives in SBUF, so the
spike threshold runs as ONE is_gt instruction per chunk over [128, tc*1024].
Spikes are written as uint8 {0,1} (exact) to quarter the output HBM traffic;
the host converts back to f32.  x-in DMAs issue on the SP HW-DGE ring and
o-out DMAs on the ACT ring (first chunk's two x slices go to both rings so
the fill isn't serialized), with 4/3/4-deep tile pools for full overlap.

Per step the membrane update is one fused DVE op:
    u_t = select(VTH >= u_{t-1}, u_{t-1}, 0) * TAU + x_t
which reproduces the reference rounding exactly: round(TAU*u) then *{0,1}
then round(+x) == round(TAU*(u*{0,1})) + x for each branch.  The spike
compare is a strict is_gt (no activation-table approximations anywhere).

All compute is on the Vector (DVE) engine; nothing runs on gpsimd (Q7
software loops are ~15ns/element - two orders of magnitude off DVE).
"""

import os
import numpy as np

import concourse.bass as bass
import concourse.bacc as bacc
import concourse.tile as tile
from concourse import mybir
from concourse.bass_utils import run_bass_kernel_spmd

TAU = 0.3
VTH = 0.3

T = 50
S_FULL = 16 * 64 * 32 * 32          # 1,048,576 spatial elements
N_CORES = 8
S_CORE = S_FULL // N_CORES          # 131,072
P = 128                             # SBUF partitions
F = S_CORE // P                     # 1024 spatial elements per partition

TC = int(os.environ.get("LIF_TC", "2"))             # time-steps per chunk
NC = T // TC                                        # chunks (must divide T)
SPIKE_CHUNK = os.environ.get("LIF_SPIKE_CHUNK", "1") == "1"
O_DT = os.environ.get("LIF_O_DT", "u8")             # u8 | bf16 | f32
X_BUFS = int(os.environ.get("LIF_X_BUFS", "4"))
U_BUFS = int(os.environ.get("LIF_U_BUFS", "3"))
O_BUFS = int(os.environ.get("LIF_O_BUFS", "4"))
# DMA issue queues: sync | scalar | alt (alternate per chunk across both
# HW-DGE rings so neither sequencer saturates)
DMA_Q = os.environ.get("LIF_DMA_Q", "dir")
X_SPLIT = int(os.environ.get("LIF_X_SPLIT", "2"))   # x-DMA slices per chunk
O_SPLIT = int(os.environ.get("LIF_O_SPLIT", "1"))   # o-DMA slices per chunk
# Shorten the serial end-of-kernel tail (last x-DMA -> fused -> spike -> o-DMA)
# by finishing with single-step chunks.
TAIL1 = int(os.environ.get("LIF_TAIL1", "2"))       # trailing TC=1 chunks
assert T % TC == 0 and TC % X_SPLIT == 0 and TC % O_SPLIT == 0
assert TAIL1 % TC == 0 and TAIL1 < T
# chunk schedule: uniform TC chunks, then TAIL1 single-step chunks
CHUNKS = [TC] * ((T - TAIL1) // TC) + [1] * TAIL1
N2 = (T - TAIL1) // TC                              # count of TC-sized chunks

# results of the last run (for test.py to inspect trace/exec time)
LAST_RESULTS = None

_FUSED_OP = None


def _get_fused_op():
    """Register the fused gated-leak op: out = select(VTH >= u, u, 0)*TAU + x.

    One DVE instruction per scan step instead of two scalar_tensor_tensor
    passes.  Registered at runtime into concourse.dve_ops' module-level
    registry (OPS / CUSTOM_DVE_SPECS / opcode map), which is all the
    table-gen path reads."""
    global _FUSED_OP
    if _FUSED_OP is not None:
        return _FUSED_OP
    import concourse.dve_ops as dve_ops
    from concourse.dve_spec import Spec, Src0, Src1, C0, C1, Zero, select, lower
    from concourse.dve_uop import DveOpSpec

    name = "LIF_GATED_LEAK_ANT"
    spec = Spec(
        body=select(C0 >= Src0, Src0, Zero) * C1 + Src1,
        reference=lambda in0, in1, s0, s1, imm2: (
            np.where(s0 >= in0, in0, np.float32(0.0)).astype(np.float32) * np.float32(s1)
        ).astype(np.float32)
        + in1,
    )
    existing = {op.name for op in dve_ops.OPS}
    if name not in existing:
        row = dve_ops._CUSTOM_DVE_ROW_BASE + len(dve_ops.OPS)
        assert row < 0x20, "custom-DVE opcode row overflow"
        # pin the sha to what lower() actually produces (self-consistent)
        shas = {}
        for ver in ("v3", "v4"):
            uops = lower(spec, ver=ver)
            shas[ver] = DveOpSpec(name=name, opcode=row, uops=uops, rd1_en=True).sha(ver)
        op = dve_ops.DveOp(name, spec, subdim=False, uops_sha=shas)
        dve_ops.OPS.append(op)
        dve_ops.CUSTOM_DVE_SPECS[name] = spec
        dve_ops._SUB_OPCODE_FOR_NAME[name] = row
        _FUSED_OP = op
    else:
        _FUSED_OP = next(op for op in dve_ops.OPS if op.name == name)
    return _FUSED_OP


def _o_mybir_dt():
    return {
        "u8": mybir.dt.uint8,
        "bf16": mybir.dt.bfloat16,
        "f32": mybir.dt.float32,
    }[O_DT]


def _build_program():
    f32 = mybir.dt.float32
    odt = _o_mybir_dt()
    nc = bacc.Bacc("TRN2", target_bir_lowering=False, debug=False)

    x_d2 = nc.dram_tensor("x2", [N2, P, TC, F], f32, kind="ExternalInput").ap()
    o_d2 = nc.dram_tensor("o2", [N2, P, TC, F], odt, kind="ExternalOutput").ap()
    if TAIL1:
        x_d1 = nc.dram_tensor("x1", [TAIL1, P, 1, F], f32, kind="ExternalInput").ap()
        o_d1 = nc.dram_tensor("o1", [TAIL1, P, 1, F], odt, kind="ExternalOutput").ap()

    fused = _get_fused_op()

    with tile.TileContext(nc) as tc:
        with (
            tc.tile_pool(name="xp", bufs=X_BUFS) as xp,
            tc.tile_pool(name="up", bufs=U_BUFS) as up,
            tc.tile_pool(name="op", bufs=O_BUFS) as op_,
        ):
            def dma_eng(idx, out=False):
                if DMA_Q == "sync":
                    return nc.sync
                if DMA_Q == "scalar":
                    return nc.scalar
                if DMA_Q == "dir":  # x-in on SP ring, o-out on ACT ring
                    return nc.scalar if out else nc.sync
                return nc.sync if idx % 2 == 0 else nc.scalar

            u_prev = None  # [P, F] slice of the previous chunk's history
            for c, tcn in enumerate(CHUNKS[:N2]):
                xin = x_d2[c]
                xt = xp.tile([P, tcn, F], f32)
                nspl = X_SPLIT if tcn % X_SPLIT == 0 else 1
                xs = tcn // nspl
                for s in range(nspl):
                    # first chunk: land the two slices via both HW-DGE rings
                    # concurrently so fill isn't serialized on one ring
                    eng = (nc.sync if s == 0 else nc.scalar) if c == 0 \
                        else dma_eng(c)
                    eng.dma_start(
                        out=xt[:, s * xs:(s + 1) * xs, :],
                        in_=xin[:, s * xs:(s + 1) * xs, :],
                    )
                uh = up.tile([P, tcn, F], f32)  # membrane history for chunk
                ot = op_.tile([P, tcn, F], odt)

                for tl in range(tcn):
                    u_new = uh[:, tl, :]
                    if c == 0 and tl == 0:
                        # u_0 = x_0 (zero carry)
                        nc.vector.tensor_copy(u_new, xt[:, 0, :])
                    else:
                        nc.vector._custom_dve(
                            fused,
                            out=u_new,
                            in0=u_prev,
                            in1=xt[:, tl, :],
                            s0=VTH,
                            s1=TAU,
                        )
                    u_prev = u_new
                    if not SPIKE_CHUNK:
                        nc.vector.tensor_scalar(
                            ot[:, tl, :], u_new, VTH, None, mybir.AluOpType.is_gt
                        )
                if SPIKE_CHUNK:
                    # one strict-compare over the whole chunk history
                    nc.vector.tensor_scalar(
                        ot[:], uh[:], VTH, None, mybir.AluOpType.is_gt
                    )
                nspl = O_SPLIT if tcn % O_SPLIT == 0 else 1
                os_ = tcn // nspl
                for s in range(nspl):
                    dma_eng(c + 1, out=True).dma_start(
                        out=o_d2[c][:, s * os_:(s + 1) * os_, :],
                        in_=ot[:, s * os_:(s + 1) * os_, :],
                    )

            # --- tail: TAIL1 single-step chunks, emitted with all fused ops
            # BEFORE their spikes so the scheduler keeps the serial
            # fused(k)->fused(k+1) chain on the critical path and slots the
            # (off-path) spikes/o-DMAs into the gaps after it.
            if TAIL1:
                txts, tuhs, tots = [], [], []
                for k in range(TAIL1):
                    xt = xp.tile([P, 1, F], f32)
                    nc.sync.dma_start(out=xt[:], in_=x_d1[k])
                    txts.append(xt)
                for k in range(TAIL1):
                    uh = up.tile([P, 1, F], f32)
                    nc.vector._custom_dve(
                        fused,
                        out=uh[:, 0, :],
                        in0=u_prev,
                        in1=txts[k][:, 0, :],
                        s0=VTH,
                        s1=TAU,
                    )
                    u_prev = uh[:, 0, :]
                    tuhs.append(uh)
                for k in range(TAIL1):
                    ot = op_.tile([P, 1, F], odt)
                    nc.vector.tensor_scalar(
                        ot[:], tuhs[k][:], VTH, None, mybir.AluOpType.is_gt
                    )
                    tots.append(ot)
                for k in range(TAIL1):
                    # last o-DMA on the (idle, lower-DGE-latency) SP ring so
                    # its issue isn't queued behind the previous o-DMA's
                    # in-order sem-wait on the ACT sequencer
                    eng = nc.sync if k == TAIL1 - 1 else nc.scalar
                    eng.dma_start(out=o_d1[k], in_=tots[k][:])
    nc.compile()
    return nc


def kernel(x, ksi=None, trace=False):
    """Full-input entry: x [16,64,32,32,50] f32 -> spikes, same shape.
    (ksi is unused by the reference computation.)"""
    global LAST_RESULTS
    x = np.ascontiguousarray(np.asarray(x, dtype=np.float32))
    orig_shape = x.shape
    xf = x.reshape(S_FULL, T)

    nc = _build_program()

    # device layout per core: [chunk, partition, t-in-chunk, free-spatial]
    T2 = T - TAIL1
    in_maps = []
    for i in range(N_CORES):
        xc = xf[i * S_CORE:(i + 1) * S_CORE]            # [S_CORE, T]
        x2 = xc[:, :T2].reshape(P, F, N2, TC).transpose(2, 0, 3, 1)
        m = {"x2": np.ascontiguousarray(x2)}            # [N2, P, TC, F]
        if TAIL1:
            x1 = xc[:, T2:].reshape(P, F, TAIL1, 1).transpose(2, 0, 3, 1)
            m["x1"] = np.ascontiguousarray(x1)          # [TAIL1, P, 1, F]
        in_maps.append(m)

    res = run_bass_kernel_spmd(nc, in_maps, list(range(N_CORES)), trace=trace)
    LAST_RESULTS = res

    def decode(r, nch, tcn, tspan):
        oc = np.asarray(r).transpose(1, 3, 0, 2).reshape(S_CORE, tspan)
        if oc.dtype != np.float32:
            oc = (oc != 0).astype(np.float32) if O_DT == "u8" else oc.astype(np.float32)
        return oc

    out = np.empty((S_FULL, T), dtype=np.float32)
    for i in range(N_CORES):
        sl = out[i * S_CORE:(i + 1) * S_CORE]
        sl[:, :T2] = decode(res.results[i]["o2"], N2, TC, T2)
        if TAIL1:
            sl[:, T2:] = decode(res.results[i]["o1"], TAIL1, 1, TAIL1)
    return out.reshape(orig_shape)


# revision 27
# speedup vs baseline: 1.0062x; 1.0042x over previous
"""LIF spike-train scan (nn_LIFSpike) on 8 TRN2 NeuronCores.

Reference semantics (fp32, bit-exact):
    u_t = TAU * u_{t-1} * (1 - o_{t-1}) + x_t ;  o_t = (u_t > VTH)
with u_{-1} = o_{-1} = 0, scanned over the trailing time dim (T=50).

Sharding: pure data parallel - the 16*64*32*32 = 1,048,576 spatial elements
split evenly across 8 cores (131,072 = 128 partitions x 1024 each).

On-chip layout per core: the time axis is chunked (TC-step chunks, ending
with TAIL1 single-step chunks that shorten the serial end-of-kernel tail);
each chunk tile is [128 partitions, tc, 1024] so every compute instruction
covers the full 1024-element free dim (amortizes the cayman per-instruction
read-write bubble).  The membrane history for a chunk lives in SBUF, so the
spike threshold runs as ONE is_gt instruction per chunk over [128, tc*1024].
Spikes are written as uint8 {0,1} (exact) to quarter the output HBM traffic;
the host converts back to f32.  x-in DMAs issue on the SP HW-DGE ring and
o-out DMAs on the ACT ring (first chunk's two x slices go to both rings so
the fill isn't serialized), with 4/3/4-deep tile pools for full overlap.

Per step the membrane update is one fused DVE op:
    u_t = select(VTH >= u_{t-1}, u_{t-1}, 0) * TAU + x_t
which reproduces the reference rounding exactly: round(TAU*u) then *{0,1}
then round(+x) == round(TAU*(u*{0,1})) + x for each branch.  The spike
compare is a strict is_gt (no activation-table approximations anywhere).

All compute is on the Vector (DVE) engine; nothing runs on gpsimd (Q7
software loops are ~15ns/element - two orders of magnitude off DVE).
"""

import os
import numpy as np

import concourse.bass as bass
import concourse.bacc as bacc
import concourse.tile as tile
from concourse import mybir
from concourse.bass_utils import run_bass_kernel_spmd

TAU = 0.3
VTH = 0.3

T = 50
S_FULL = 16 * 64 * 32 * 32          # 1,048,576 spatial elements
N_CORES = 8
S_CORE = S_FULL // N_CORES          # 131,072
P = 128                             # SBUF partitions
F = S_CORE // P                     # 1024 spatial elements per partition

TC = int(os.environ.get("LIF_TC", "2"))             # time-steps per chunk
NC = T // TC                                        # chunks (must divide T)
SPIKE_CHUNK = os.environ.get("LIF_SPIKE_CHUNK", "1") == "1"
O_DT = os.environ.get("LIF_O_DT", "u8")             # u8 | bf16 | f32
X_BUFS = int(os.environ.get("LIF_X_BUFS", "4"))
U_BUFS = int(os.environ.get("LIF_U_BUFS", "3"))
O_BUFS = int(os.environ.get("LIF_O_BUFS", "4"))
# DMA issue queues: sync | scalar | alt (alternate per chunk across both
# HW-DGE rings so neither sequencer saturates)
DMA_Q = os.environ.get("LIF_DMA_Q", "dir")
X_SPLIT = int(os.environ.get("LIF_X_SPLIT", "2"))   # x-DMA slices per chunk
O_SPLIT = int(os.environ.get("LIF_O_SPLIT", "1"))   # o-DMA slices per chunk
# Shorten the serial end-of-kernel tail (last x-DMA -> fused -> spike -> o-DMA)
# by finishing with single-step chunks.
TAIL1 = int(os.environ.get("LIF_TAIL1", "2"))       # trailing TC=1 chunks
# Spike/output granularity: SG chunks share one uh/o super-tile, one is_gt and
# one o-DMA (fewer instructions, bigger o transfers).  Applies to the leading
# floor-multiple of SG chunks; the remainder + tail keep per-chunk granularity
# so the endgame chain stays short.
SG = int(os.environ.get("LIF_SG", "2"))             # chunks per spike group
assert T % TC == 0 and TC % X_SPLIT == 0 and TC % O_SPLIT == 0
assert TAIL1 % TC == 0 and TAIL1 < T
# chunk schedule: uniform TC chunks, then TAIL1 single-step chunks
CHUNKS = [TC] * ((T - TAIL1) // TC) + [1] * TAIL1
N2 = (T - TAIL1) // TC                              # count of TC-sized chunks

# results of the last run (for test.py to inspect trace/exec time)
LAST_RESULTS = None

_FUSED_OP = None


def _get_fused_op():
    """Register the fused gated-leak op: out = select(VTH >= u, u, 0)*TAU + x.

    One DVE instruction per scan step instead of two scalar_tensor_tensor
    passes.  Registered at runtime into concourse.dve_ops' module-level
    registry (OPS / CUSTOM_DVE_SPECS / opcode map), which is all the
    table-gen path reads."""
    global _FUSED_OP
    if _FUSED_OP is not None:
        return _FUSED_OP
    import concourse.dve_ops as dve_ops
    from concourse.dve_spec import Spec, Src0, Src1, C0, C1, Zero, select, lower
    from concourse.dve_uop import DveOpSpec

    name = "LIF_GATED_LEAK_ANT"
    spec = Spec(
        body=select(C0 >= Src0, Src0, Zero) * C1 + Src1,
        reference=lambda in0, in1, s0, s1, imm2: (
            np.where(s0 >= in0, in0, np.float32(0.0)).astype(np.float32) * np.float32(s1)
        ).astype(np.float32)
        + in1,
    )
    existing = {op.name for op in dve_ops.OPS}
    if name not in existing:
        row = dve_ops._CUSTOM_DVE_ROW_BASE + len(dve_ops.OPS)
        assert row < 0x20, "custom-DVE opcode row overflow"
        # pin the sha to what lower() actually produces (self-consistent)
        shas = {}
        for ver in ("v3", "v4"):
            uops = lower(spec, ver=ver)
            shas[ver] = DveOpSpec(name=name, opcode=row, uops=uops, rd1_en=True).sha(ver)
        op = dve_ops.DveOp(name, spec, subdim=False, uops_sha=shas)
        dve_ops.OPS.append(op)
        dve_ops.CUSTOM_DVE_SPECS[name] = spec
        dve_ops._SUB_OPCODE_FOR_NAME[name] = row
        _FUSED_OP = op
    else:
        _FUSED_OP = next(op for op in dve_ops.OPS if op.name == name)
    return _FUSED_OP


def _o_mybir_dt():
    return {
        "u8": mybir.dt.uint8,
        "bf16": mybir.dt.bfloat16,
        "f32": mybir.dt.float32,
    }[O_DT]


def _build_program():
    f32 = mybir.dt.float32
    odt = _o_mybir_dt()
    nc = bacc.Bacc("TRN2", target_bir_lowering=False, debug=False)

    assert N2 % SG == 0
    SGT = SG * TC                     # steps per spike group
    x_d2 = nc.dram_tensor("x2", [N2, P, TC, F], f32, kind="ExternalInput").ap()
    o_d2 = nc.dram_tensor("o2", [N2 // SG, P, SGT, F], odt, kind="ExternalOutput").ap()
    if TAIL1:
        x_d1 = nc.dram_tensor("x1", [TAIL1, P, 1, F], f32, kind="ExternalInput").ap()
        o_d1 = nc.dram_tensor("o1", [TAIL1, P, 1, F], odt, kind="ExternalOutput").ap()

    fused = _get_fused_op()

    with tile.TileContext(nc) as tc:
        with (
            tc.tile_pool(name="xp", bufs=X_BUFS) as xp,
            tc.tile_pool(name="up", bufs=U_BUFS) as up,
            tc.tile_pool(name="op", bufs=O_BUFS) as op_,
        ):
            def dma_eng(idx, out=False):
                if DMA_Q == "sync":
                    return nc.sync
                if DMA_Q == "scalar":
                    return nc.scalar
                if DMA_Q == "dir":  # x-in on SP ring, o-out on ACT ring
                    return nc.scalar if out else nc.sync
                return nc.sync if idx % 2 == 0 else nc.scalar

            u_prev = None  # [P, F] slice of the previous chunk's history
            for g in range(N2 // SG):
                uh = up.tile([P, SGT, F], f32)  # group membrane history
                ot = op_.tile([P, SGT, F], odt)
                for ci in range(SG):
                    c = g * SG + ci
                    xin = x_d2[c]
                    xt = xp.tile([P, TC, F], f32)
                    nspl = X_SPLIT if TC % X_SPLIT == 0 else 1
                    xs = TC // nspl
                    for s in range(nspl):
                        # first chunk: land the two slices via both HW-DGE
                        # rings concurrently so fill isn't serialized
                        eng = (nc.sync if s == 0 else nc.scalar) if c == 0 \
                            else dma_eng(c)
                        eng.dma_start(
                            out=xt[:, s * xs:(s + 1) * xs, :],
                            in_=xin[:, s * xs:(s + 1) * xs, :],
                        )
                    for tl in range(TC):
                        j = ci * TC + tl
                        u_new = uh[:, j, :]
                        if c == 0 and tl == 0:
                            # u_0 = x_0 (zero carry)
                            nc.vector.tensor_copy(u_new, xt[:, 0, :])
                        else:
                            nc.vector._custom_dve(
                                fused,
                                out=u_new,
                                in0=u_prev,
                                in1=xt[:, tl, :],
                                s0=VTH,
                                s1=TAU,
                            )
                        u_prev = u_new
                        if not SPIKE_CHUNK:
                            nc.vector.tensor_scalar(
                                ot[:, j, :], u_new, VTH, None,
                                mybir.AluOpType.is_gt,
                            )
                if SPIKE_CHUNK:
                    # one strict-compare over the whole group history
                    nc.vector.tensor_scalar(
                        ot[:], uh[:], VTH, None, mybir.AluOpType.is_gt
                    )
                nspl = O_SPLIT if SGT % O_SPLIT == 0 else 1
                os_ = SGT // nspl
                for s in range(nspl):
                    dma_eng(g + 1, out=True).dma_start(
                        out=o_d2[g][:, s * os_:(s + 1) * os_, :],
                        in_=ot[:, s * os_:(s + 1) * os_, :],
                    )

            # --- tail: TAIL1 single-step chunks, emitted with all fused ops
            # BEFORE their spikes so the scheduler keeps the serial
            # fused(k)->fused(k+1) chain on the critical path and slots the
            # (off-path) spikes/o-DMAs into the gaps after it.
            if TAIL1:
                txts, tuhs, tots = [], [], []
                for k in range(TAIL1):
                    xt = xp.tile([P, 1, F], f32)
                    nc.sync.dma_start(out=xt[:], in_=x_d1[k])
                    txts.append(xt)
                for k in range(TAIL1):
                    uh = up.tile([P, 1, F], f32)
                    nc.vector._custom_dve(
                        fused,
                        out=uh[:, 0, :],
                        in0=u_prev,
                        in1=txts[k][:, 0, :],
                        s0=VTH,
                        s1=TAU,
                    )
                    u_prev = uh[:, 0, :]
                    tuhs.append(uh)
                for k in range(TAIL1):
                    ot = op_.tile([P, 1, F], odt)
                    nc.vector.tensor_scalar(
                        ot[:], tuhs[k][:], VTH, None, mybir.AluOpType.is_gt
                    )
                    tots.append(ot)
                for k in range(TAIL1):
                    # last o-DMA on the (idle, lower-DGE-latency) SP ring so
                    # its issue isn't queued behind the previous o-DMA's
                    # in-order sem-wait on the ACT sequencer
                    eng = nc.sync if k == TAIL1 - 1 else nc.scalar
                    eng.dma_start(out=o_d1[k], in_=tots[k][:])
    nc.compile()
    return nc


def kernel(x, ksi=None, trace=False):
    """Full-input entry: x [16,64,32,32,50] f32 -> spikes, same shape.
    (ksi is unused by the reference computation.)"""
    global LAST_RESULTS
    x = np.ascontiguousarray(np.asarray(x, dtype=np.float32))
    orig_shape = x.shape
    xf = x.reshape(S_FULL, T)

    nc = _build_program()

    # device layout per core: [chunk, partition, t-in-chunk, free-spatial]
    T2 = T - TAIL1
    in_maps = []
    for i in range(N_CORES):
        xc = xf[i * S_CORE:(i + 1) * S_CORE]            # [S_CORE, T]
        x2 = xc[:, :T2].reshape(P, F, N2, TC).transpose(2, 0, 3, 1)
        m = {"x2": np.ascontiguousarray(x2)}            # [N2, P, TC, F]
        if TAIL1:
            x1 = xc[:, T2:].reshape(P, F, TAIL1, 1).transpose(2, 0, 3, 1)
            m["x1"] = np.ascontiguousarray(x1)          # [TAIL1, P, 1, F]
        in_maps.append(m)

    res = run_bass_kernel_spmd(nc, in_maps, list(range(N_CORES)), trace=trace)
    LAST_RESULTS = res

    def decode(r, nch, tcn, tspan):
        oc = np.asarray(r).transpose(1, 3, 0, 2).reshape(S_CORE, tspan)
        if oc.dtype != np.float32:
            oc = (oc != 0).astype(np.float32) if O_DT == "u8" else oc.astype(np.float32)
        return oc

    out = np.empty((S_FULL, T), dtype=np.float32)
    for i in range(N_CORES):
        sl = out[i * S_CORE:(i + 1) * S_CORE]
        sl[:, :T2] = decode(res.results[i]["o2"], N2, TC, T2)
        if TAIL1:
            sl[:, T2:] = decode(res.results[i]["o1"], TAIL1, 1, TAIL1)
    return out.reshape(orig_shape)
